# revision 7
# baseline (speedup 1.0000x reference)
"""Trainium2 Bass kernel for nn_DotProductAttention_76338748719461.

Attention with a multiplicative mask and softmax over the QUERY axis
(axis=1 of [B, Lq, Lk] scores):

    S[b,q,k]  = (Q[b,q,:] . K[b,k,:]) / 8 + max(log(mask[0,q,k]), F32_MIN)
    A         = softmax(S, axis=q)
    out[b,q,v]= sum_k A[b,q,k] * V[b,k,v]

Strategy (per NeuronCore; batch is data-parallel over 8 cores, 2 per core):
  * Work in the TRANSPOSED score orientation S_T[k, q] so the softmax
    reduction (over q) is a free-axis reduction, fused into the Exp
    activation via accum_out.
  * S_T = (identity @ logm_T) + KT.T @ (Q/8)T accumulated in PSUM, where
    logm_T = log(mask.T + 1e-38) in fp16, produced once per core via
    ACT log + PE block transposes.
  * PM_T = exp(S_T) (ACT, PSUM->SBUF) with accum_out giving the softmax
    denominator D[k] per partition row.
  * out_T[v, q] = sum_k (V[k,v]/D[k]) . PM_T[k, q] via PE accumulation,
    then PE-transposed back to [q, v] and DMA'd out.
"""

import os
import numpy as np

B, LQ, LK, D, DV = 16, 2048, 2048, 64, 64
NCORES = 8
BPC = B // NCORES  # batches per core
P = 128
CH = 512  # matmul moving chunk (one PSUM bank of fp32)
HALF = 1024  # exp granularity (half a k-tile row)
NT_Q = LQ // P  # 16
NT_K = LK // P  # 16
SCALE = 1.0 / 8.0  # 1/sqrt(64)
LOG_BIAS = 1e-38  # keeps log(0) finite (-87.5); matches reference clamp behavior

# knobs
MM_DT = os.environ.get("MM_DT", "float32r")  # dtype for QK / AV matmuls


_CACHED = None


def _build_module():
    import concourse.bass as bass
    import concourse.mybir as mybir
    import concourse.tile as tile
    from concourse import bacc
    from concourse.bass import ds, ts
    from concourse.masks import make_identity
    from contextlib import ExitStack

    f32 = mybir.dt.float32
    f16 = mybir.dt.float16
    mm_dt = getattr(mybir.dt, MM_DT)
    AF = mybir.ActivationFunctionType

    nc = bacc.Bacc("TRN2", target_bir_lowering=False, debug=False)
    q_d = nc.dram_tensor("q", [BPC, LQ, D], f32, kind="ExternalInput").ap()
    k_d = nc.dram_tensor("k", [BPC, LK, D], f32, kind="ExternalInput").ap()
    v_d = nc.dram_tensor("v", [BPC, LK, DV], f32, kind="ExternalInput").ap()
    m_d = nc.dram_tensor("m", [LQ, LK], f32, kind="ExternalInput").ap()
    o_d = nc.dram_tensor("o", [BPC, LQ, DV], f32, kind="ExternalOutput").ap()

    def mm(ap):
        return ap.bitcast(mm_dt) if MM_DT != "float32" else ap

    with tile.TileContext(nc) as tc, ExitStack() as ctx:
        consts = ctx.enter_context(tc.tile_pool(name="consts", bufs=1))
        ident32 = consts.tile([P, P], f32)
        make_identity(nc, ident32)
        ident16 = consts.tile([P, P], f16)
        make_identity(nc, ident16)
        log_bias = consts.tile([P, 1], f32)
        nc.gpsimd.memset(log_bias[:], LOG_BIAS)

        big = ctx.enter_context(tc.tile_pool(name="big", bufs=1))
        logm_T = big.tile([P, NT_K, LQ], f16)  # [k%128, k//128, q]
        QT = big.tile([D, BPC, LQ], f32)  # Q^T / 8
        KT = big.tile([D, BPC, LK], f32)
        v_nat = big.tile([P, BPC, NT_K, DV], f32)

        nc.sync.dma_start(
            v_nat[:], v_d.rearrange("b (t p) d -> p b t d", p=P)
        )

        # ---------- prep phase: mask -> logm_T; Q,K -> QT,KT ----------
        with ExitStack() as prep:
            mask_nat = prep.enter_context(tc.tile_pool(name="mask_nat", bufs=2))
            logm_nat = prep.enter_context(tc.tile_pool(name="logm_nat", bufs=2))
            qk_nat = prep.enter_context(tc.tile_pool(name="qk_nat", bufs=2))
            psum_mt = prep.enter_context(
                tc.tile_pool(name="psum_mt", bufs=2, space="PSUM")
            )
            psum_qt = prep.enter_context(
                tc.tile_pool(name="psum_qt", bufs=2, space="PSUM")
            )

            # Q/K transposes (both batches)
            for b in range(BPC):
                for t_ap, dram, scale in ((QT, q_d, SCALE), (KT, k_d, None)):
                    nat = qk_nat.tile([P, NT_Q, D], f32, tag="nat")
                    nc.sync.dma_start(
                        nat[:], dram[b].rearrange("(t p) d -> p t d", p=P)
                    )
                    for g in range(NT_Q // 4):  # groups of 4 transposes
                        pq = psum_qt.tile([D, 4 * P], f32)
                        for u in range(4):
                            t = 4 * g + u
                            nc.tensor.transpose(
                                pq[:, ds(P * u, P)], nat[:, t, :], ident32
                            )
                        dst = t_ap[:, b, ds(4 * P * g, 4 * P)]
                        if scale is not None:
                            nc.vector.tensor_scalar_mul(dst, pq[:], scale)
                        else:
                            nc.vector.tensor_copy(dst, pq[:])

            # mask -> log -> block-transpose -> logm_T (fp16)
            for i in range(NT_Q):
                mt = mask_nat.tile([P, LK], f32)
                nc.sync.dma_start(mt[:], m_d[ds(P * i, P), :])
                lt = logm_nat.tile([P, LK], f16)
                nc.scalar.activation(lt[:], mt[:], AF.Ln, bias=log_bias[:])
                pmt = psum_mt.tile([P, LK], f16)
                for j in range(NT_K):
                    nc.tensor.transpose(
                        pmt[:, ds(P * j, P)], lt[:, ds(P * j, P)], ident16
                    )
                nc.vector.tensor_copy(
                    logm_T[:, :, ds(P * i, P)],
                    pmt[:].rearrange("p (j q) -> p j q", q=P),
                )

        # ---------- main loop ----------
        psum_s = ctx.enter_context(tc.tile_pool(name="psum_s", bufs=2, space="PSUM"))
        psum_o = ctx.enter_context(tc.tile_pool(name="psum_o", bufs=1, space="PSUM"))
        work = ctx.enter_context(tc.tile_pool(name="work", bufs=2))
        outp = ctx.enter_context(tc.tile_pool(name="outp", bufs=2))

        for b in range(BPC):
            O_ps = psum_o.tile([DV, LQ], f32, tag="o")
            pending_av = None  # (Vp, PM, j)

            for j in range(NT_K):
                # S_T tile in PSUM: two halves of [P, HALF]
                Sh = [
                    psum_s.tile([P, HALF], f32, tag="s", name=f"s{h}")
                    for h in range(2)
                ]
                for h in range(2):
                    for c in range(2):
                        nc.tensor.matmul(
                            Sh[h][:, ts(c, CH)],
                            ident16,
                            logm_T[:, j, ds(HALF * h + CH * c, CH)],
                            start=True,
                            stop=False,
                        )
                for h in range(2):
                    for c in range(2):
                        nc.tensor.matmul(
                            Sh[h][:, ts(c, CH)],
                            mm(KT[:, b, ds(P * j, P)]),
                            mm(QT[:, b, ds(HALF * h + CH * c, CH)]),
                            start=False,
                            stop=True,
                        )

                # deferred AV of previous k-tile keeps PE busy while exp runs
                if pending_av is not None:
                    pVp, pPM, pj = pending_av
                    for c in range(LQ // CH):
                        nc.tensor.matmul(
                            O_ps[:, ts(c, CH)],
                            mm(pVp[:]),
                            mm(pPM[:, ts(c, CH)]),
                            start=(pj == 0),
                            stop=(pj == NT_K - 1),
                        )

                PM = work.tile([P, LQ], f32, tag="pm")
                D2 = work.tile([P, 2], f32, tag="d2")
                for h in range(2):
                    nc.scalar.activation(
                        PM[:, ds(HALF * h, HALF)],
                        Sh[h][:],
                        AF.Exp,
                        accum_out=D2[:, ds(h, 1)],
                    )
                Dsum = work.tile([P, 1], f32, tag="dsum")
                nc.vector.reduce_sum(Dsum[:], D2[:], axis=mybir.AxisListType.X)
                R = work.tile([P, 1], f32, tag="r")
                nc.vector.reciprocal(R[:], Dsum[:])
                Vp = work.tile([P, DV], f32, tag="vp")
                nc.vector.tensor_scalar_mul(Vp[:], v_nat[:, b, j, :], R[:])
                pending_av = (Vp, PM, j)

            pVp, pPM, pj = pending_av
            for c in range(LQ // CH):
                nc.tensor.matmul(
                    O_ps[:, ts(c, CH)],
                    mm(pVp[:]),
                    mm(pPM[:, ts(c, CH)]),
                    start=(pj == 0),
                    stop=(pj == NT_K - 1),
                )

            # evacuate + transpose back to [q, v]
            OT = outp.tile([DV, LQ], f32, tag="ot")
            nc.vector.tensor_copy(OT[:], O_ps[:])
            out_sb = outp.tile([P, NT_Q, DV], f32, tag="osb")
            for g in range(NT_Q // 8):
                tp = psum_o.tile([P, 8 * DV], f32, tag="o")
                for u in range(8):
                    t = 8 * g + u
                    nc.tensor.transpose(
                        tp[:, ds(DV * u, DV)],
                        OT[:, ds(P * t, P)],
                        ident32[0:DV, 0:DV],
                    )
                nc.vector.tensor_copy(
                    out_sb[:, ds(8 * g, 8), :],
                    tp[:].rearrange("p (t d) -> p t d", d=DV),
                )
            nc.sync.dma_start(
                o_d[b].rearrange("(t p) d -> p t d", p=P), out_sb[:]
            )

    nc.compile()
    return nc


def _get_module():
    global _CACHED
    if _CACHED is None:
        _CACHED = _build_module()
    return _CACHED


def kernel(query, key, value, mask, _trace=False):
    from concourse.bass_utils import run_bass_kernel_spmd

    query = np.asarray(query, dtype=np.float32)
    key = np.asarray(key, dtype=np.float32)
    value = np.asarray(value, dtype=np.float32)
    mask = np.asarray(mask, dtype=np.float32)

    nc = _get_module()
    in_maps = [
        {
            "q": query[c * BPC : (c + 1) * BPC],
            "k": key[c * BPC : (c + 1) * BPC],
            "v": value[c * BPC : (c + 1) * BPC],
            "m": mask[0],
        }
        for c in range(NCORES)
    ]
    res = run_bass_kernel_spmd(
        nc, in_maps, core_ids=list(range(NCORES)), trace=_trace
    )
    out = np.concatenate([res.results[c]["o"] for c in range(NCORES)], axis=0)
    if _trace:
        return out, res
    return out
